# revision 6
# baseline (speedup 1.0000x reference)
"""Trainium2 Bass kernel for nn_DFMBitFlipPredictor (dense-graph GNN message passing).

Math (per batch b, layer l):
  pre[i,j,:] = ai[i,:] + aj[j,:] + J[i,j]*We[:] + b1          ai = h@Wi, aj = h@Wj
  agg        = (sum_j silu(pre)) @ msg_w2 + n*msg_b2          (matmul pulled out of the j-sum)
  h          = FiLM(h + silu(h@Ua + agg@Ub + ub1) @ upd_w2 + ub2)
  rates      = softplus(silu(h@ro_w1+ro_b1)@ro_w2 + ro_b2)

Device strategy: 8 cores = 4 batches x 2 receiver-halves.  Per (k in H):
  psum[i,j] = J[i,j] + aj[j,k]/We[k]      (two matmuls: identity-copy of J + rank-1 row bcast)
  S[i,k]    = sum_j silu(We[k]*psum + (ai+b1)[i,k])   -- ONE ScalarE op: scale, per-partition
              bias, and the j-reduce (accum_out) all fused.
The /We[k] trick folds the per-k edge scale into the activation's scale operand; the fp32
error of the rescale is ~|pre|*eps regardless of We magnitude.  All weight-only reshapes
(Wj/We, msg_w2@Ub, FiLM constants, global embedding) are precomputed on host.
Per-layer the two cores of a batch exchange the transposed partial S via a pair AllGather.
SPMD: one program for all cores; per-core specialization only through input data
(J rows, half-selection matrices Sel0/Sel1).
"""

import os
import sys

for _p in ("/opt/trn_rl_repo", "/root/.axon_site/_ro/trn_rl_repo"):
    if os.path.isdir(_p) and _p not in sys.path:
        sys.path.insert(0, _p)

import numpy as np

import concourse.bacc as bacc
import concourse.mybir as mybir
from concourse import tile
from concourse.bass_utils import run_bass_kernel_spmd

N_CORES = 8
B, N, H, L = 4, 256, 128, 4
F32 = mybir.dt.float32
AF = mybir.ActivationFunctionType
ALU = mybir.AluOpType


def build_nc():
    nc = bacc.Bacc("TRN2", target_bir_lowering=False, debug=False, num_devices=N_CORES)

    # ---- I/O ----
    d_hT0 = nc.dram_tensor("hT0", [H, N], F32, kind="ExternalInput")
    d_jown = nc.dram_tensor("jown", [128, N], F32, kind="ExternalInput")
    d_sel0 = nc.dram_tensor("sel0", [128, 128], F32, kind="ExternalInput")
    d_sel1 = nc.dram_tensor("sel1", [128, 128], F32, kind="ExternalInput")
    d_eye = nc.dram_tensor("eye", [128, 128], F32, kind="ExternalInput")
    # per-layer weight stacks (L, 128, 128)
    d_wjw = nc.dram_tensor("wjw", [L, H, H], F32, kind="ExternalInput")
    d_wi = nc.dram_tensor("wi", [L, H, H], F32, kind="ExternalInput")
    d_ua = nc.dram_tensor("ua", [L, H, H], F32, kind="ExternalInput")
    d_w2u = nc.dram_tensor("w2u", [L, H, H], F32, kind="ExternalInput")
    d_uw2 = nc.dram_tensor("uw2", [L, H, H], F32, kind="ExternalInput")
    d_werep = nc.dram_tensor("werep", [L, H, H], F32, kind="ExternalInput")
    d_b1rep = nc.dram_tensor("b1rep", [L, H, H], F32, kind="ExternalInput")
    # per-layer column vectors, stored (128, L)
    d_bu = nc.dram_tensor("bu", [H, L], F32, kind="ExternalInput")
    d_g1 = nc.dram_tensor("g1", [H, L], F32, kind="ExternalInput")
    d_cf = nc.dram_tensor("cf", [H, L], F32, kind="ExternalInput")
    # readout
    d_row1 = nc.dram_tensor("row1", [H, H], F32, kind="ExternalInput")
    d_rob1 = nc.dram_tensor("rob1", [H, 1], F32, kind="ExternalInput")
    d_row2 = nc.dram_tensor("row2", [H, 1], F32, kind="ExternalInput")
    d_rob2 = nc.dram_tensor("rob2", [1, 1], F32, kind="ExternalInput")
    d_out = nc.dram_tensor("rates", [1, N], F32, kind="ExternalOutput")

    with tile.TileContext(nc) as tc:
        with (
            tc.tile_pool(name="wpool", bufs=1) as wp,
            tc.tile_pool(name="work", bufs=2) as wk,
            tc.tile_pool(name="scr", bufs=1) as scrp,
            tc.tile_pool(name="ps", bufs=3, space="PSUM") as ps,
            tc.tile_pool(name="dram", bufs=2, space="DRAM") as dp,
        ):
            # ---- load constants / weights ----
            jown = wp.tile([128, N], F32)
            nc.sync.dma_start(jown[:], d_jown[:])
            sel0 = wp.tile([128, 128], F32)
            nc.sync.dma_start(sel0[:], d_sel0[:])
            sel1 = wp.tile([128, 128], F32)
            nc.sync.dma_start(sel1[:], d_sel1[:])
            eye = wp.tile([128, 128], F32)
            nc.sync.dma_start(eye[:], d_eye[:])

            def load_stack(name, dram):
                t = wp.tile([H, L * H], F32, name=name)
                nc.sync.dma_start(
                    t.rearrange("p (l f) -> p l f", f=H),
                    dram.rearrange("l p f -> p l f"),
                )
                return t

            wjw = load_stack("wjw_sb", d_wjw)
            wi = load_stack("wi_sb", d_wi)
            ua = load_stack("ua_sb", d_ua)
            w2u = load_stack("w2u_sb", d_w2u)
            uw2 = load_stack("uw2_sb", d_uw2)
            werep = load_stack("werep_sb", d_werep)
            b1rep = load_stack("b1rep_sb", d_b1rep)

            bu = wp.tile([H, L], F32)
            nc.sync.dma_start(bu[:], d_bu[:])
            g1 = wp.tile([H, L], F32)
            nc.sync.dma_start(g1[:], d_g1[:])
            cf = wp.tile([H, L], F32)
            nc.sync.dma_start(cf[:], d_cf[:])
            row1 = wp.tile([H, H], F32)
            nc.sync.dma_start(row1[:], d_row1[:])
            rob1 = wp.tile([H, 1], F32)
            nc.sync.dma_start(rob1[:], d_rob1[:])
            row2 = wp.tile([H, 1], F32)
            nc.sync.dma_start(row2[:], d_row2[:])
            rob2 = wp.tile([1, 1], F32)
            nc.sync.dma_start(rob2[:], d_rob2[:])

            scr = scrp.tile([128, N], F32)  # silu bulk output (discarded)

            hT = wk.tile([H, N], F32, tag="hT")
            nc.sync.dma_start(hT[:], d_hT0[:])

            for l in range(L):
                lw = slice(l * H, (l + 1) * H)
                # aj/We in (k, j) layout: AJW = (Wj/We)^T @ hT
                p_ajw = ps.tile([H, N], F32, tag="pmed", name="p_ajw", bufs=2)
                nc.tensor.matmul(p_ajw[:], wjw[:, lw], hT[:], start=True, stop=True)
                ajw = wk.tile([H, N], F32, tag="ajw", name="ajw")
                nc.vector.tensor_copy(ajw[:], p_ajw[:])

                # bias s_own[io, k] = (h_own @ Wi + b1)  via both halves + Sel
                p_s0 = ps.tile([128, H], F32, tag="psm", name="p_s0", bufs=2)
                nc.tensor.matmul(p_s0[:], hT[:, 0:128], wi[:, lw], start=True, stop=True)
                s0 = wk.tile([128, H], F32, tag="s0", name="s0")
                nc.vector.tensor_copy(s0[:], p_s0[:])
                p_s1 = ps.tile([128, H], F32, tag="psm", name="p_s1", bufs=2)
                nc.tensor.matmul(p_s1[:], hT[:, 128:256], wi[:, lw], start=True, stop=True)
                s1 = wk.tile([128, H], F32, tag="s1", name="s1")
                nc.vector.tensor_copy(s1[:], p_s1[:])
                p_sown = ps.tile([128, H], F32, tag="psm", name="p_sown", bufs=2)
                nc.tensor.matmul(p_sown[:], sel0[:], s0[:], start=True, stop=False)
                nc.tensor.matmul(p_sown[:], sel1[:], s1[:], start=False, stop=True)
                sown = wk.tile([128, H], F32, tag="sown", name="sown")
                nc.vector.tensor_add(sown[:], p_sown[:], b1rep[:, lw])

                # the big sweep: S[io, k] = sum_j silu(We[k]*(J + aj/We) + s_own[:,k])
                S = wk.tile([128, H], F32, tag="S", name="S")
                for k in range(H):
                    pe = ps.tile([128, N], F32, tag="pe", name="pe", bufs=3)
                    nc.tensor.matmul(
                        pe[:], eye[:, k : k + 1].broadcast_to([128, 128]), ajw[:],
                        start=True, stop=False,
                    )
                    nc.tensor.matmul(pe[:], eye[:], jown[:], start=False, stop=True)
                    nc.scalar.activation(
                        scr[:],
                        pe[:],
                        AF.Silu,
                        bias=sown[:, k : k + 1],
                        scale=werep[:, l * H + k : l * H + k + 1],
                        accum_out=S[:, k : k + 1],
                    )

                # transpose S -> (k, io), exchange with pair core -> STfull (k, 256)
                p_st = ps.tile([128, H], F32, tag="psm", name="p_st", bufs=2)
                nc.tensor.transpose(p_st[:], S[:], eye[:])
                st_own = wk.tile([H, 128], F32, tag="st_own", name="st_own")
                nc.vector.tensor_copy(st_own[:], p_st[:])

                cc_in = dp.tile([H, 128], F32, tag="cc_in", name="cc_in")
                cc_out = dp.tile([2 * H, 128], F32, tag="cc_out", name="cc_out")
                nc.gpsimd.dma_start(cc_in[:], st_own[:])
                nc.gpsimd.collective_compute(
                    "AllGather",
                    ALU.bypass,
                    replica_groups=[[0, 1], [2, 3], [4, 5], [6, 7]],
                    ins=[cc_in.opt()],
                    outs=[cc_out.opt()],
                )
                stfull = wk.tile([H, N], F32, tag="stfull", name="stfull")
                nc.sync.dma_start(stfull[:, 0:128], cc_out[0:128, :])
                nc.sync.dma_start(stfull[:, 128:256], cc_out[128:256, :])

                # node update (transposed layout, full 256 nodes on both cores)
                p_u = ps.tile([H, N], F32, tag="pmed", name="p_u", bufs=2)
                nc.tensor.matmul(p_u[:], ua[:, lw], hT[:], start=True, stop=False)
                nc.tensor.matmul(p_u[:], w2u[:, lw], stfull[:], start=False, stop=True)
                uT = wk.tile([H, N], F32, tag="uT", name="uT")
                nc.scalar.activation(uT[:], p_u[:], AF.Silu, bias=bu[:, l : l + 1])
                p_d = ps.tile([H, N], F32, tag="pmed", name="p_d", bufs=2)
                nc.tensor.matmul(p_d[:], uw2[:, lw], uT[:], start=True, stop=True)
                hsum = wk.tile([H, N], F32, tag="hsum", name="hsum")
                nc.vector.tensor_add(hsum[:], p_d[:], hT[:])
                hT = wk.tile([H, N], F32, tag="hT", name="hT")
                nc.vector.tensor_scalar(
                    hT[:], hsum[:], g1[:, l : l + 1], cf[:, l : l + 1], ALU.mult, ALU.add
                )

            # readout
            p_z = ps.tile([H, N], F32, tag="pmed", name="p_z", bufs=2)
            nc.tensor.matmul(p_z[:], row1[:], hT[:], start=True, stop=True)
            zT = wk.tile([H, N], F32, tag="zT", name="zT")
            nc.scalar.activation(zT[:], p_z[:], AF.Silu, bias=rob1[:, 0:1])
            p_r = ps.tile([1, N], F32, tag="psm", name="p_r", bufs=2)
            nc.tensor.matmul(p_r[:], row2[:], zT[:], start=True, stop=True)
            # softplus(x) = max(x,0) + ln(1+exp(-|x|)); x = p_r + ro_b2
            x_sb = wk.tile([1, N], F32, tag="x_sb", name="x_sb")
            nc.scalar.activation(x_sb[:], p_r[:], AF.Identity, bias=rob2[0:1, 0:1])
            ax = wk.tile([1, N], F32, tag="ax", name="ax")
            nc.scalar.activation(ax[:], x_sb[:], AF.Abs)
            en = wk.tile([1, N], F32, tag="en", name="en")
            nc.scalar.activation(en[:], ax[:], AF.Exp, scale=-1.0)
            ep1 = wk.tile([1, N], F32, tag="ep1", name="ep1")
            nc.vector.tensor_scalar_add(ep1[:], en[:], 1.0)
            l1p = wk.tile([1, N], F32, tag="l1p", name="l1p")
            nc.scalar.activation(l1p[:], ep1[:], AF.Ln)
            rx = wk.tile([1, N], F32, tag="rx", name="rx")
            nc.vector.tensor_scalar_max(rx[:], x_sb[:], 0.0)
            rates_sb = wk.tile([1, N], F32, tag="rates_sb", name="rates_sb")
            nc.vector.tensor_add(rates_sb[:], rx[:], l1p[:])
            nc.sync.dma_start(d_out[:], rates_sb[:])

    nc.compile()
    return nc


def _silu(x):
    return x / (1.0 + np.exp(-x))


def make_in_maps(inputs):
    x_t = np.asarray(inputs["x_t"], np.float32)
    t = np.asarray(inputs["t"], np.float32)
    beta = np.asarray(inputs["beta"], np.float32)
    J = np.asarray(inputs["J_mat"], np.float32)
    h_field = np.asarray(inputs["h_field"], np.float32)
    npw = np.asarray(inputs["node_proj_w"], np.float32)
    npb = np.asarray(inputs["node_proj_b"], np.float32)
    msg_w1 = np.asarray(inputs["msg_w1"], np.float32)
    msg_b1 = np.asarray(inputs["msg_b1"], np.float32)
    msg_w2 = np.asarray(inputs["msg_w2"], np.float32)
    msg_b2 = np.asarray(inputs["msg_b2"], np.float32)
    upd_w1 = np.asarray(inputs["upd_w1"], np.float32)
    upd_b1 = np.asarray(inputs["upd_b1"], np.float32)
    upd_w2 = np.asarray(inputs["upd_w2"], np.float32)
    upd_b2 = np.asarray(inputs["upd_b2"], np.float32)
    film_w = np.asarray(inputs["film_w"], np.float32)
    film_b = np.asarray(inputs["film_b"], np.float32)

    # host precompute
    feats = np.stack([x_t, np.broadcast_to(h_field[None, :], x_t.shape)], axis=-1)
    h0 = feats @ npw + npb  # (B, N, H)
    g = np.concatenate([t, beta], axis=-1)  # (B, 2)
    ge_w1 = np.asarray(inputs["ge_w1"], np.float32)
    ge_b1 = np.asarray(inputs["ge_b1"], np.float32)
    ge_w2 = np.asarray(inputs["ge_w2"], np.float32)
    ge_b2 = np.asarray(inputs["ge_b2"], np.float32)
    gemb = _silu(g @ ge_w1 + ge_b1) @ ge_w2 + ge_b2  # (B, GD)
    fb = np.einsum("bg,lgh->blh", gemb, film_w) + film_b  # (B, L, 2H)
    gamma, shift = fb[..., :H], fb[..., H:]
    g1 = (1.0 + gamma).astype(np.float32)  # (B, L, H)
    cf = (upd_b2[None] * (1.0 + gamma) + shift).astype(np.float32)

    Wi = msg_w1[:, :H, :]
    Wj = msg_w1[:, H : 2 * H, :]
    We = msg_w1[:, 2 * H, :]  # (L, H)
    We_safe = np.where(We == 0.0, np.float32(1e-30), We)
    wjw = (Wj / We_safe[:, None, :]).astype(np.float32)
    werep = np.broadcast_to(We[:, None, :], (L, H, H)).astype(np.float32)
    b1rep = np.broadcast_to(msg_b1[:, None, :], (L, H, H)).astype(np.float32)
    Ua = upd_w1[:, :H, :]
    Ub = upd_w1[:, H:, :]
    w2u = np.einsum("lkh,lhc->lkc", msg_w2, Ub).astype(np.float32)
    bu = (np.einsum("lh,lhc->lc", N * msg_b2, Ub) + upd_b1).astype(np.float32)  # (L, H)

    c = np.ascontiguousarray
    common = {
        "eye": c(np.eye(128, dtype=np.float32)),
        "wjw": c(wjw),
        "wi": c(Wi),
        "ua": c(Ua),
        "w2u": c(w2u),
        "uw2": c(upd_w2),
        "werep": c(werep),
        "b1rep": c(b1rep),
        "bu": c(bu.T),
        "row1": c(np.asarray(inputs["ro_w1"], np.float32)),
        "rob1": c(np.asarray(inputs["ro_b1"], np.float32).reshape(H, 1)),
        "row2": c(np.asarray(inputs["ro_w2"], np.float32).reshape(H, 1)),
        "rob2": c(np.asarray(inputs["ro_b2"], np.float32).reshape(1, 1)),
    }
    zeros128 = np.zeros((128, 128), np.float32)
    in_maps = []
    for core in range(N_CORES):
        b, ih = core // 2, core % 2
        m = dict(common)
        m["hT0"] = c(h0[b].T)
        m["jown"] = c(J[ih * 128 : (ih + 1) * 128, :])
        m["sel0"] = common["eye"] if ih == 0 else zeros128
        m["sel1"] = common["eye"] if ih == 1 else zeros128
        m["g1"] = c(g1[b].T)
        m["cf"] = c(cf[b].T)
        in_maps.append(m)
    return in_maps


_CACHE = {}


def _get_nc():
    if "nc" not in _CACHE:
        _CACHE["nc"] = build_nc()
    return _CACHE["nc"]


def _run(nc, in_maps, **kwargs):
    res = run_bass_kernel_spmd(nc, in_maps, core_ids=list(range(N_CORES)), **kwargs)
    return res.results


def kernel(**inputs):
    nc = _get_nc()
    in_maps = make_in_maps(inputs)
    results = _run(nc, in_maps)
    out = np.zeros((B, N), np.float32)
    for b in range(B):
        out[b] = results[2 * b]["rates"][0]
    return out





# revision 9
# speedup vs baseline: 4.4402x; 4.4402x over previous
"""Trainium2 Bass kernel for nn_DFMBitFlipPredictor (dense-graph GNN message passing).

Math (per batch b, layer l):
  pre[i,j,:] = ai[i,:] + aj[j,:] + J[i,j]*We[:] + b1          ai = h@Wi, aj = h@Wj
  agg        = (sum_j silu(pre)) @ msg_w2 + n*msg_b2          (matmul pulled out of the j-sum)
  h          = FiLM(h + silu(h@Ua + agg@Ub + ub1) @ upd_w2 + ub2)
  rates      = softplus(silu(h@ro_w1+ro_b1)@ro_w2 + ro_b2)

Device strategy: 8 cores = 4 batches x 2 receiver-halves; one SPMD program, per-core
specialization only through input data (J rows, half-selection matrices Sel0/Sel1).
Layout: partitions = k (the H dim), free = (i_chunk, j 256).  Per chunk:
  DMA broadcasts J rows across the 128 k-partitions,
  scalar_tensor_tensor:  J*We[k] + aj[j,k]      (per-partition scalar We)
  gpsimd add:            + ai[i,k]              (broadcast APs over j)
  ScalarE Silu with per-partition bias b1[k],
  segmented tensor_reduce over innermost j -> ST[k, i] columns.
Per-layer the two cores of a batch exchange transposed partial sums ST via a pair
AllGather; the node-update then runs identically (full 256 nodes) on both cores.
All weight-only reshapes (msg_w2@Ub, FiLM constants, global embedding, h0) are
precomputed on host in kernel().  Bulk ops with big free dims keep both the
instruction count (~60/layer) and per-engine work low.
"""

import os
import sys

for _p in ("/opt/trn_rl_repo", "/root/.axon_site/_ro/trn_rl_repo"):
    if os.path.isdir(_p) and _p not in sys.path:
        sys.path.insert(0, _p)

import numpy as np

import concourse.bacc as bacc
import concourse.mybir as mybir
from concourse import tile
from concourse.bass_utils import run_bass_kernel_spmd

N_CORES = 8
B, N, H, L = 4, 256, 128, 4
IC = 16  # receiver rows per chunk
NCHUNK = 128 // IC
F32 = mybir.dt.float32
AF = mybir.ActivationFunctionType
ALU = mybir.AluOpType


def build_nc(use_cc=True):
    nc = bacc.Bacc("TRN2", target_bir_lowering=False, debug=False, num_devices=N_CORES)

    # ---- I/O ----
    d_hT0 = nc.dram_tensor("hT0", [H, N], F32, kind="ExternalInput")
    d_jflat = nc.dram_tensor("jflat", [1, 128 * N], F32, kind="ExternalInput")
    d_sel0 = nc.dram_tensor("sel0", [128, 128], F32, kind="ExternalInput")
    d_sel1 = nc.dram_tensor("sel1", [128, 128], F32, kind="ExternalInput")
    # per-layer weight stacks (L, 128, 128)
    d_wj = nc.dram_tensor("wj", [L, H, H], F32, kind="ExternalInput")
    d_wi = nc.dram_tensor("wi", [L, H, H], F32, kind="ExternalInput")
    d_ua = nc.dram_tensor("ua", [L, H, H], F32, kind="ExternalInput")
    d_w2u = nc.dram_tensor("w2u", [L, H, H], F32, kind="ExternalInput")
    d_uw2 = nc.dram_tensor("uw2", [L, H, H], F32, kind="ExternalInput")
    # per-layer column vectors, stored (128, L)
    d_wecol = nc.dram_tensor("wecol", [H, L], F32, kind="ExternalInput")
    d_b1col = nc.dram_tensor("b1col", [H, L], F32, kind="ExternalInput")
    d_bu = nc.dram_tensor("bu", [H, L], F32, kind="ExternalInput")
    d_g1 = nc.dram_tensor("g1", [H, L], F32, kind="ExternalInput")
    d_cf = nc.dram_tensor("cf", [H, L], F32, kind="ExternalInput")
    # readout
    d_row1 = nc.dram_tensor("row1", [H, H], F32, kind="ExternalInput")
    d_rob1 = nc.dram_tensor("rob1", [H, 1], F32, kind="ExternalInput")
    d_row2 = nc.dram_tensor("row2", [H, 1], F32, kind="ExternalInput")
    d_rob2 = nc.dram_tensor("rob2", [1, 1], F32, kind="ExternalInput")
    d_out = nc.dram_tensor("rates", [1, N], F32, kind="ExternalOutput")

    with tile.TileContext(nc) as tc:
        with (
            tc.tile_pool(name="wpool", bufs=1) as wp,
            tc.tile_pool(name="work", bufs=2) as wk,
            tc.tile_pool(name="big", bufs=2) as bp,
            tc.tile_pool(name="ps", bufs=2, space="PSUM") as ps,
            tc.tile_pool(name="dram", bufs=2, space="DRAM") as dp,
        ):
            # ---- load constants / weights ----
            sel0 = wp.tile([128, 128], F32)
            nc.sync.dma_start(sel0[:], d_sel0[:])
            sel1 = wp.tile([128, 128], F32)
            nc.sync.dma_start(sel1[:], d_sel1[:])

            def load_stack(name, dram):
                t = wp.tile([H, L * H], F32, name=name)
                nc.sync.dma_start(
                    t.rearrange("p (l f) -> p l f", f=H),
                    dram.rearrange("l p f -> p l f"),
                )
                return t

            wj = load_stack("wj_sb", d_wj)
            wi = load_stack("wi_sb", d_wi)
            ua = load_stack("ua_sb", d_ua)
            w2u = load_stack("w2u_sb", d_w2u)
            uw2 = load_stack("uw2_sb", d_uw2)

            def load_cols(name, dram):
                t = wp.tile([H, L], F32, name=name)
                nc.sync.dma_start(t[:], dram[:])
                return t

            wecol = load_cols("wecol_sb", d_wecol)
            b1col = load_cols("b1col_sb", d_b1col)
            bu = load_cols("bu_sb", d_bu)
            g1 = load_cols("g1_sb", d_g1)
            cf = load_cols("cf_sb", d_cf)

            row1 = wp.tile([H, H], F32)
            nc.sync.dma_start(row1[:], d_row1[:])
            rob1 = wp.tile([H, 1], F32)
            nc.sync.dma_start(rob1[:], d_rob1[:])
            row2 = wp.tile([H, 1], F32)
            nc.sync.dma_start(row2[:], d_row2[:])
            rob2 = wp.tile([1, 1], F32)
            nc.sync.dma_start(rob2[:], d_rob2[:])

            hT = wk.tile([H, N], F32, tag="hT")
            nc.sync.dma_start(hT[:], d_hT0[:])

            for l in range(L):
                lw = slice(l * H, (l + 1) * H)
                lc = slice(l, l + 1)
                # aj in (k, j) layout: AJ = Wj^T @ hT
                p_aj = ps.tile([H, N], F32, tag="pmed", name="p_aj")
                nc.tensor.matmul(p_aj[:], wj[:, lw], hT[:], start=True, stop=True)
                aj_sb = wk.tile([H, N], F32, tag="aj_sb", name="aj_sb")
                nc.vector.tensor_copy(aj_sb[:], p_aj[:])

                # sT_own[k, io] = (h_own @ Wi)^T via half-products + Sel
                p_s0 = ps.tile([128, H], F32, tag="psm", name="p_s0")
                nc.tensor.matmul(p_s0[:], hT[:, 0:128], wi[:, lw], start=True, stop=True)
                s0 = wk.tile([128, H], F32, tag="s0", name="s0")
                nc.vector.tensor_copy(s0[:], p_s0[:])
                p_s1 = ps.tile([128, H], F32, tag="psm", name="p_s1")
                nc.tensor.matmul(p_s1[:], hT[:, 128:256], wi[:, lw], start=True, stop=True)
                s1 = wk.tile([128, H], F32, tag="s1", name="s1")
                nc.vector.tensor_copy(s1[:], p_s1[:])
                p_sT = ps.tile([128, H], F32, tag="psm", name="p_sT")
                nc.tensor.matmul(p_sT[:], s0[:], sel0[:], start=True, stop=False)
                nc.tensor.matmul(p_sT[:], s1[:], sel1[:], start=False, stop=True)
                sT = wk.tile([H, 128], F32, tag="sT", name="sT")
                nc.vector.tensor_copy(sT[:], p_sT[:])

                # big sweep in chunks of (IC receivers x 256 senders)
                st_own = wk.tile([H, 128], F32, tag="st_own", name="st_own")
                for c in range(NCHUNK):
                    cs = slice(c * IC, (c + 1) * IC)
                    jrep = bp.tile([128, IC * N], F32, tag="jrep", name="jrep")
                    nc.sync.dma_start(
                        jrep.rearrange("p (i j) -> p i j", j=N),
                        d_jflat[0:1, c * IC * N : (c + 1) * IC * N]
                        .rearrange("a (i j) -> a i j", j=N)
                        .broadcast_to([128, IC, N]),
                    )
                    scr1 = bp.tile([128, IC * N], F32, tag="scr1", name="scr1")
                    nc.vector.scalar_tensor_tensor(
                        scr1.rearrange("p (i j) -> p i j", j=N),
                        jrep.rearrange("p (i j) -> p i j", j=N),
                        wecol[:, lc],
                        aj_sb.unsqueeze(1).broadcast_to([128, IC, N]),
                        ALU.mult,
                        ALU.add,
                    )
                    scr2 = bp.tile([128, IC * N], F32, tag="scr2", name="scr2")
                    nc.gpsimd.tensor_add(
                        scr2.rearrange("p (i j) -> p i j", j=N),
                        scr1.rearrange("p (i j) -> p i j", j=N),
                        sT[:, cs].unsqueeze(2).broadcast_to([128, IC, N]),
                    )
                    sil = bp.tile([128, IC * N], F32, tag="sil", name="sil")
                    nc.scalar.activation(sil[:], scr2[:], AF.Silu, bias=b1col[:, lc])
                    nc.vector.tensor_reduce(
                        st_own[:, cs],
                        sil.rearrange("p (i j) -> p i j", j=N),
                        axis=mybir.AxisListType.X,
                        op=ALU.add,
                    )

                # exchange ST halves with pair core
                cc_in = dp.tile([H, 128], F32, tag="cc_in", name="cc_in")
                cc_out = dp.tile([2 * H, 128], F32, tag="cc_out", name="cc_out")
                nc.gpsimd.dma_start(cc_in[:], st_own[:])
                if use_cc:
                    nc.gpsimd.collective_compute(
                        "AllGather",
                        ALU.bypass,
                        replica_groups=[[0, 1], [2, 3], [4, 5], [6, 7]],
                        ins=[cc_in.opt()],
                        outs=[cc_out.opt()],
                    )
                else:  # timing-only stand-in
                    nc.gpsimd.dma_start(cc_out[0:128, :], cc_in[:])
                    nc.gpsimd.dma_start(cc_out[128:256, :], cc_in[:])
                stfull = wk.tile([H, N], F32, tag="stfull", name="stfull")
                nc.sync.dma_start(stfull[:, 0:128], cc_out[0:128, :])
                nc.sync.dma_start(stfull[:, 128:256], cc_out[128:256, :])

                # node update (transposed layout, full 256 nodes on both cores)
                p_u = ps.tile([H, N], F32, tag="pmed", name="p_u")
                nc.tensor.matmul(p_u[:], ua[:, lw], hT[:], start=True, stop=False)
                nc.tensor.matmul(p_u[:], w2u[:, lw], stfull[:], start=False, stop=True)
                uT = wk.tile([H, N], F32, tag="uT", name="uT")
                nc.scalar.activation(uT[:], p_u[:], AF.Silu, bias=bu[:, lc])
                p_d = ps.tile([H, N], F32, tag="pmed", name="p_d")
                nc.tensor.matmul(p_d[:], uw2[:, lw], uT[:], start=True, stop=True)
                hsum = wk.tile([H, N], F32, tag="hsum", name="hsum")
                nc.vector.tensor_add(hsum[:], p_d[:], hT[:])
                hT = wk.tile([H, N], F32, tag="hT", name="hT")
                nc.vector.tensor_scalar(
                    hT[:], hsum[:], g1[:, lc], cf[:, lc], ALU.mult, ALU.add
                )

            # readout
            p_z = ps.tile([H, N], F32, tag="pmed", name="p_z")
            nc.tensor.matmul(p_z[:], row1[:], hT[:], start=True, stop=True)
            zT = wk.tile([H, N], F32, tag="zT", name="zT")
            nc.scalar.activation(zT[:], p_z[:], AF.Silu, bias=rob1[:, 0:1])
            p_r = ps.tile([1, N], F32, tag="psm", name="p_r")
            nc.tensor.matmul(p_r[:], row2[:], zT[:], start=True, stop=True)
            # softplus(x) = max(x,0) + ln(1+exp(-|x|)); x = p_r + ro_b2
            x_sb = wk.tile([1, N], F32, tag="x_sb", name="x_sb")
            nc.scalar.activation(x_sb[:], p_r[:], AF.Identity, bias=rob2[0:1, 0:1])
            ax = wk.tile([1, N], F32, tag="ax", name="ax")
            nc.scalar.activation(ax[:], x_sb[:], AF.Abs)
            en = wk.tile([1, N], F32, tag="en", name="en")
            nc.scalar.activation(en[:], ax[:], AF.Exp, scale=-1.0)
            ep1 = wk.tile([1, N], F32, tag="ep1", name="ep1")
            nc.vector.tensor_scalar_add(ep1[:], en[:], 1.0)
            l1p = wk.tile([1, N], F32, tag="l1p", name="l1p")
            nc.scalar.activation(l1p[:], ep1[:], AF.Ln)
            rx = wk.tile([1, N], F32, tag="rx", name="rx")
            nc.vector.tensor_scalar_max(rx[:], x_sb[:], 0.0)
            rates_sb = wk.tile([1, N], F32, tag="rates_sb", name="rates_sb")
            nc.vector.tensor_add(rates_sb[:], rx[:], l1p[:])
            nc.sync.dma_start(d_out[:], rates_sb[:])

    nc.compile()
    return nc


def _silu(x):
    return x / (1.0 + np.exp(-x))


def make_in_maps(inputs):
    x_t = np.asarray(inputs["x_t"], np.float32)
    t = np.asarray(inputs["t"], np.float32)
    beta = np.asarray(inputs["beta"], np.float32)
    J = np.asarray(inputs["J_mat"], np.float32)
    h_field = np.asarray(inputs["h_field"], np.float32)
    npw = np.asarray(inputs["node_proj_w"], np.float32)
    npb = np.asarray(inputs["node_proj_b"], np.float32)
    msg_w1 = np.asarray(inputs["msg_w1"], np.float32)
    msg_b1 = np.asarray(inputs["msg_b1"], np.float32)
    msg_w2 = np.asarray(inputs["msg_w2"], np.float32)
    msg_b2 = np.asarray(inputs["msg_b2"], np.float32)
    upd_w1 = np.asarray(inputs["upd_w1"], np.float32)
    upd_b1 = np.asarray(inputs["upd_b1"], np.float32)
    upd_w2 = np.asarray(inputs["upd_w2"], np.float32)
    upd_b2 = np.asarray(inputs["upd_b2"], np.float32)
    film_w = np.asarray(inputs["film_w"], np.float32)
    film_b = np.asarray(inputs["film_b"], np.float32)

    # host precompute
    feats = np.stack([x_t, np.broadcast_to(h_field[None, :], x_t.shape)], axis=-1)
    h0 = feats @ npw + npb  # (B, N, H)
    g = np.concatenate([t, beta], axis=-1)  # (B, 2)
    ge_w1 = np.asarray(inputs["ge_w1"], np.float32)
    ge_b1 = np.asarray(inputs["ge_b1"], np.float32)
    ge_w2 = np.asarray(inputs["ge_w2"], np.float32)
    ge_b2 = np.asarray(inputs["ge_b2"], np.float32)
    gemb = _silu(g @ ge_w1 + ge_b1) @ ge_w2 + ge_b2  # (B, GD)
    fb = np.einsum("bg,lgh->blh", gemb, film_w) + film_b  # (B, L, 2H)
    gamma, shift = fb[..., :H], fb[..., H:]
    g1 = (1.0 + gamma).astype(np.float32)  # (B, L, H)
    cf = (upd_b2[None] * (1.0 + gamma) + shift).astype(np.float32)

    Wi = msg_w1[:, :H, :]
    Wj = msg_w1[:, H : 2 * H, :]
    We = msg_w1[:, 2 * H, :]  # (L, H)
    Ua = upd_w1[:, :H, :]
    Ub = upd_w1[:, H:, :]
    w2u = np.einsum("lkh,lhc->lkc", msg_w2, Ub).astype(np.float32)
    bu = (np.einsum("lh,lhc->lc", N * msg_b2, Ub) + upd_b1).astype(np.float32)  # (L, H)

    c = np.ascontiguousarray
    common = {
        "wj": c(Wj),
        "wi": c(Wi),
        "ua": c(Ua),
        "w2u": c(w2u),
        "uw2": c(upd_w2),
        "wecol": c(We.T),
        "b1col": c(msg_b1.T),
        "bu": c(bu.T),
        "row1": c(np.asarray(inputs["ro_w1"], np.float32)),
        "rob1": c(np.asarray(inputs["ro_b1"], np.float32).reshape(H, 1)),
        "row2": c(np.asarray(inputs["ro_w2"], np.float32).reshape(H, 1)),
        "rob2": c(np.asarray(inputs["ro_b2"], np.float32).reshape(1, 1)),
    }
    eye = np.eye(128, dtype=np.float32)
    zeros128 = np.zeros((128, 128), np.float32)
    in_maps = []
    for core in range(N_CORES):
        b, ih = core // 2, core % 2
        m = dict(common)
        m["hT0"] = c(h0[b].T)
        m["jflat"] = c(J[ih * 128 : (ih + 1) * 128, :].reshape(1, 128 * N))
        m["sel0"] = eye if ih == 0 else zeros128
        m["sel1"] = eye if ih == 1 else zeros128
        m["g1"] = c(g1[b].T)
        m["cf"] = c(cf[b].T)
        in_maps.append(m)
    return in_maps


_CACHE = {}


def _get_nc():
    if "nc" not in _CACHE:
        _CACHE["nc"] = build_nc()
    return _CACHE["nc"]


def _run(nc, in_maps, **kwargs):
    res = run_bass_kernel_spmd(nc, in_maps, core_ids=list(range(N_CORES)), **kwargs)
    return res.results


def kernel(**inputs):
    nc = _get_nc()
    in_maps = make_in_maps(inputs)
    results = _run(nc, in_maps)
    out = np.zeros((B, N), np.float32)
    for b in range(B):
        out[b] = results[2 * b]["rates"][0]
    return out
